# revision 47
# baseline (speedup 1.0000x reference)
"""DigitCaps routing kernel for TRN2 (8 NeuronCores, SPMD data-parallel over batch).

Problem: inputs [64, 4096, 8] f32, W [4096, 10, 8, 16] f32.
  u_hat[b,i,n,d] = sum_p inputs[b,i,p] * W[i,n,p,d]
  3 dynamic-routing iterations (softmax over n, weighted sum over i, squash,
  agreement update), output v [64, 10, 1, 16] f32.

Sharding: batch 64 -> 8 cores x 8 samples. W replicated (streamed once per core).
All on-device data is f32: the routing fixed-point amplifies input rounding by
~100x, so fp16/bf16 anywhere on the value path fails accuracy.

Per-core device layout (sigma = i'*8 + b, where i' = i mod 16, b = local sample):
  U  [128=(i'*8+b), 256 chunks * 160]  f32   u_hat, chunk k holds i in [16k,16k+16)
  L  [128, 256*10] f32                       routing logits
Production: per chunk one matmul  lhsT = Xall_k [128=(i'*8+p), 128=(i'*8+b)]
  (block-diagonal x, prebuilt on host), rhs = W2_k [128=(i'*8+p), 160=(n,d)].
s-phase: per chunk matmul with lhsT = C_all [128, 80=(n,b)] (masked softmax
  weights) accumulating PSUM [80, 160]; diagonal extracted with 10 tiny DMAs.
Logit update: DVE mult by broadcast v + d-tree reduction, all f32.
"""

from contextlib import ExitStack

import numpy as np

import concourse.bass as bass
import concourse.tile as tile
from concourse import bacc, mybir
from concourse.tile import TileContext

AF = mybir.ActivationFunctionType
ALU = mybir.AluOpType

N_CORES = 8
B_FULL = 64
I_FULL = 4096
P_DIM = 8          # Din
N_CAP = 10
D_CAP = 16
ND = N_CAP * D_CAP  # 160
EPS = 1e-7
ROUTING_ITERS = 3

F32 = mybir.dt.float32


def build_nc(I_dim=I_FULL, b_shard=8, phases="all"):
    """Build the single-core Bass program (SPMD: same program on all cores).

    phases: "all" | "prod" (production only) | "it0" (prod + uniform iter) |
            "s1" (.. + first s-iteration) — for cost attribution.
    """
    CH = I_dim // 16          # chunks of 16 capsules
    SUPC = 4                  # chunks per DMA superchunk
    SUP = CH // SUPC
    GRP = min(8, CH)          # chunks per group for DVE staging
    NG = CH // GRP

    # tiny SWDGE ring: all our DMAs go through HWDGE (sync/scalar)
    nc = bacc.Bacc(dynamic_dma_scratch_size=1024)

    w2_d = nc.dram_tensor("w2", [CH * 128, ND], F32, kind="ExternalInput")
    # xpk[q=(i'*8+p), k*8+b] = x[b, 16k+i', p] — compact per-chunk x operand
    xpk_d = nc.dram_tensor("xpk", [128, CH * 8], F32, kind="ExternalInput")
    # xmask[q, m=(i2*8+b)] = 1 if i2 == q//8 else 0 (block-diagonal selector)
    xmask_d = nc.dram_tensor("xmask", [128, 128], F32, kind="ExternalInput")
    mask0_d = nc.dram_tensor("mask0", [128, 8], F32, kind="ExternalInput")
    maskc_d = nc.dram_tensor("maskc", [128, 80], F32, kind="ExternalInput")
    e8_d = nc.dram_tensor("e8", [8, 128], F32, kind="ExternalInput")
    out_d = nc.dram_tensor("out", [b_shard, ND], F32, kind="ExternalOutput")

    with TileContext(nc) as tc, ExitStack() as ctx:
        # ---- pools ----
        pU = ctx.enter_context(tc.tile_pool(name="U", bufs=1))
        pL = ctx.enter_context(tc.tile_pool(name="L", bufs=1))
        pconst = ctx.enter_context(tc.tile_pool(name="const", bufs=1))
        pw2 = ctx.enter_context(tc.tile_pool(name="w2", bufs=2))
        pxa = ctx.enter_context(tc.tile_pool(name="xa", bufs=2))
        psm = ctx.enter_context(tc.tile_pool(name="sm", bufs=2))
        pprod = ctx.enter_context(tc.tile_pool(name="prod", bufs=2))
        pca = ctx.enter_context(tc.tile_pool(name="ca", bufs=2))
        psmall = ctx.enter_context(tc.tile_pool(name="small", bufs=1))
        ppsP = ctx.enter_context(tc.tile_pool(name="psP", bufs=3, space="PSUM"))
        ppsS = ctx.enter_context(tc.tile_pool(name="psS", bufs=1, space="PSUM"))
        ppsV = ctx.enter_context(tc.tile_pool(name="psV", bufs=1, space="PSUM"))

        # ---- persistent tiles ----
        U = pU.tile([128, CH * ND], F32)
        L = pL.tile([128, CH * N_CAP], F32)
        xpk_sb = pconst.tile([128, CH * 8], F32)
        xmask_sb = pconst.tile([128, 128], F32)
        mask0_sb = pconst.tile([128, 8], F32)
        maskc_sb = pconst.tile([128, 80], F32)
        e8_sb = pconst.tile([8, 128], F32)

        nc.sync.dma_start(xpk_sb[:], xpk_d[:])
        nc.sync.dma_start(xmask_sb[:], xmask_d[:])
        nc.sync.dma_start(mask0_sb[:], mask0_d[:])
        nc.sync.dma_start(maskc_sb[:], maskc_d[:])
        nc.sync.dma_start(e8_sb[:], e8_d[:])
        nc.vector.memset(L[:], 0.0)

        eps_b = pconst.tile([128, 1], F32)
        one_b = pconst.tile([128, 1], F32)
        nc.vector.memset(eps_b[:], EPS)
        nc.vector.memset(one_b[:], 1.0)

        # ---- phase A: u_hat production ----
        # per superchunk: DMA W2, build the block-diagonal X tile on-chip
        # (broadcast xpk against xmask), then SUPC matmuls; 3 chunk outputs
        # share one PSUM bank -> one batched copy per 3 MMs
        w2_r = w2_d.rearrange("(s c p) f -> s p c f", c=SUPC, p=128)
        xpk_r = xpk_sb.rearrange("p (s c b) -> p s c b", c=SUPC, b=8)
        ps = None
        for s in range(SUP):
            w2b = pw2.tile([128, SUPC * ND], F32)
            (nc.sync if s % 2 == 0 else nc.scalar).dma_start(
                w2b.rearrange("p (c f) -> p c f", c=SUPC), w2_r[s])
            xab = pxa.tile([128, SUPC * 128], F32)
            xeng = nc.vector
            xeng.tensor_tensor(
                xab.rearrange("p (c i b) -> p c i b", c=SUPC, b=8),
                xpk_r[:, s].unsqueeze(2).to_broadcast([128, SUPC, 16, 8]),
                xmask_sb.rearrange("p (i b) -> p i b", b=8)
                    .unsqueeze(1).to_broadcast([128, SUPC, 16, 8]),
                ALU.mult,
            )
            for c in range(SUPC):
                k = s * SUPC + c
                j = k % 3
                if j == 0:
                    ps = ppsP.tile([128, 3 * ND], F32)
                nc.tensor.matmul(
                    ps[:, j * ND:(j + 1) * ND],
                    xab[:, c * 128:(c + 1) * 128],
                    w2b[:, c * ND:(c + 1) * ND],
                    start=True, stop=True,
                )
                if j == 2 or k == CH - 1:
                    lo = k - j
                    if (k // 3) % 2 == 0:
                        nc.vector.tensor_copy(
                            U[:, lo * ND:(k + 1) * ND], ps[:, 0:(j + 1) * ND])
                    else:
                        nc.scalar.copy(
                            U[:, lo * ND:(k + 1) * ND], ps[:, 0:(j + 1) * ND])

        # ---- helpers ----
        def squash(in_ap, scale, copy_in=True):
            """v = squash(scale * in_ap), elementwise over [8, 160]."""
            if copy_in:
                s_sb = psmall.tile([8, ND], F32, tag="sq_s")
                nc.scalar.mul(s_sb[:], in_ap, scale)
            else:
                s_sb = in_ap  # already an SBUF tile, scale must be 1
            sq = psmall.tile([8, ND], F32, tag="sq_sq")
            nc.vector.tensor_mul(sq[:], s_sb[:], s_sb[:])
            den = psmall.tile([8, ND], F32, tag="sq_den")
            nc.scalar.activation(den[:], sq[:], AF.Identity, bias=one_b[0:8, :])
            nc.vector.reciprocal(den[:], den[:])
            rt = psmall.tile([8, ND], F32, tag="sq_rt")
            nc.scalar.activation(rt[:], sq[:], AF.Sqrt, bias=eps_b[0:8, :])
            nc.vector.reciprocal(rt[:], rt[:])
            nc.vector.tensor_mul(sq[:], sq[:], den[:])     # sq/(1+sq)
            nc.vector.tensor_mul(rt[:], sq[:], rt[:])      # * rsqrt(sq+eps)
            v_sb = psmall.tile([8, ND], F32, tag="sq_v")
            nc.vector.tensor_mul(v_sb[:], rt[:], s_sb[:])
            return v_sb

        def s_uniform():
            """s0 = 0.1 * sum_i u_hat -> [8, 160] psum accumulation."""
            s0_ps = ppsS.tile([8, ND], F32, tag="s_acc")
            for k in range(CH):
                nc.tensor.matmul(
                    s0_ps[:], mask0_sb[:], U[:, k * ND:(k + 1) * ND],
                    start=(k == 0), stop=(k == CH - 1),
                )
            return s0_ps

        def broadcast_v(v_sb):
            """v [8,160] -> vf [128,160] (replicated per sample block)."""
            vf_ps = ppsV.tile([128, ND], F32)
            nc.tensor.matmul(vf_ps[:], e8_sb[:], v_sb[:], start=True, stop=True)
            vf = psmall.tile([128, ND], F32, tag="vf")
            nc.vector.tensor_copy(vf[:], vf_ps[:])
            return vf

        def logit_update_group(eng, g, vf):
            """L[sigma, (k,n)] += sum_d U[sigma,(k,n,d)] * v for group g."""
            Lv = L.rearrange("p (a x) -> p a x", x=GRP * N_CAP)
            pr = pprod.tile([128, GRP * ND], F32, tag="pr")
            eng.tensor_tensor(
                pr.rearrange("p (k f) -> p k f", k=GRP),
                U[:, g * GRP * ND:(g + 1) * GRP * ND]
                    .rearrange("p (k f) -> p k f", k=GRP),
                vf[:].unsqueeze(1).to_broadcast([128, GRP, ND]),
                ALU.mult,
            )
            pv = pr.rearrange("p (a d) -> p a d", d=D_CAP)
            eng.tensor_tensor(pv[:, :, 0:8], pv[:, :, 0:8], pv[:, :, 8:16], ALU.add)
            eng.tensor_tensor(pv[:, :, 0:4], pv[:, :, 0:4], pv[:, :, 4:8], ALU.add)
            eng.tensor_tensor(pv[:, :, 0:2], pv[:, :, 0:2], pv[:, :, 2:4], ALU.add)
            eng.tensor_tensor(pv[:, :, 0:1], pv[:, :, 0:1], pv[:, :, 1:2], ALU.add)
            eng.tensor_tensor(Lv[:, g, :], Lv[:, g, :], pv[:, :, 0], ALU.add)

        def logit_update(v_sb):
            vf = broadcast_v(v_sb)
            for g in range(NG):
                eng = nc.vector
                logit_update_group(eng, g, vf)

        def softmax_smm_group(g, s_ps):
            """Group-local softmax + C_all build + the s accumulation MMs."""
            Lg = L.rearrange("p (g k n) -> p g k n", k=GRP, n=N_CAP)
            ee = psm.tile([128, GRP * N_CAP], F32, tag="ee")
            nc.scalar.activation(
                ee[:], Lg[:, g].rearrange("p k n -> p (k n)"), AF.Exp)
            eev = ee.rearrange("p (k n) -> p k n", n=N_CAP)
            den = psm.tile([128, GRP], F32, tag="den")
            nc.vector.tensor_reduce(
                den[:], eev, axis=mybir.AxisListType.X, op=ALU.add)
            rr = psm.tile([128, GRP], F32, tag="rr")
            nc.vector.reciprocal(rr[:], den[:])
            cc = psm.tile([128, GRP * N_CAP], F32, tag="cc")
            nc.vector.tensor_tensor(
                cc.rearrange("p (k n) -> p k n", n=N_CAP), eev,
                rr[:].unsqueeze(2).to_broadcast([128, GRP, N_CAP]),
                ALU.mult,
            )
            ca = pca.tile([128, GRP * 80], F32)
            nc.vector.tensor_tensor(
                ca.rearrange("p (k n b) -> p k n b", k=GRP, b=8),
                cc.rearrange("p (k n) -> p k n", n=N_CAP)
                    .unsqueeze(3).to_broadcast([128, GRP, N_CAP, 8]),
                maskc_sb.rearrange("p (n b) -> p n b", b=8)
                    .unsqueeze(1).to_broadcast([128, GRP, N_CAP, 8]),
                ALU.mult,
            )
            for kk in range(GRP):
                k = g * GRP + kk
                nc.tensor.matmul(
                    s_ps[:],
                    ca[:, kk * 80:(kk + 1) * 80],
                    U[:, k * ND:(k + 1) * ND],
                    start=(k == 0), stop=(k == CH - 1),
                )

        def s_iteration(v_prev=None):
            """If v_prev is given, fuse its logit-update per group with this
            iteration's softmax + s-matmuls (the PE chases the DVE/GPSIMD)."""
            vf = broadcast_v(v_prev) if v_prev is not None else None
            s_ps = ppsS.tile([80, ND], F32, tag="s_acc")
            for g in range(NG):
                if vf is not None:
                    eng = nc.vector
                    logit_update_group(eng, g, vf)
                softmax_smm_group(g, s_ps)
            return s_ps

        def extract_diag(s_ps):
            """[80,160] psum -> s_sb [8,160]: s_sb[b, n*16+d] = s_ps[n*8+b, n*16+d]."""
            # vf is dead once the s accumulation finished -> reuse its slot
            s80 = psmall.tile([80, ND], F32, tag="vf")
            nc.scalar.copy(s80[:], s_ps[:])
            s_sb = psmall.tile([8, ND], F32, tag="sq_s")
            for n in range(N_CAP):
                nc.sync.dma_start(
                    s_sb[0:8, n * D_CAP:(n + 1) * D_CAP],
                    s80[n * 8:(n + 1) * 8, n * D_CAP:(n + 1) * D_CAP],
                )
            return s_sb

        # ---- routing (logit-update of iter t fused into s-iteration t+1) ----
        if phases == "prod":
            v_sb = psmall.tile([8, ND], F32, tag="sq_v")
            nc.vector.tensor_copy(v_sb[:], U[0:8, 0:ND])
        elif phases == "it0":
            v_sb = squash(s_uniform()[:], 1.0)
            logit_update(v_sb)
        elif phases == "s1":
            v_sb = squash(s_uniform()[:], 1.0)
            v_sb = squash(extract_diag(s_iteration(v_sb))[:], 1.0,
                          copy_in=False)
        else:
            v_sb = squash(s_uniform()[:], 1.0)  # the 1/N is baked into mask0
            for it in range(1, ROUTING_ITERS):
                s_sb = extract_diag(s_iteration(v_sb))
                v_sb = squash(s_sb[:], 1.0, copy_in=False)

        nc.sync.dma_start(out_d[:], v_sb[:])

    nc.compile()
    if not nc.is_finalized():
        nc.finalize()
    return nc


# ------------------------- host-side data prep -------------------------

def prep_core_inputs(x_shard, I_dim=I_FULL):
    """Per-core xpk from x_shard [8, I, 8] f32."""
    CH = I_dim // 16
    b_shard = x_shard.shape[0]
    assert b_shard == 8

    # xs[b, k, i', p] -> xpk[(i'*8+p), k*8+b]
    xs = x_shard.reshape(b_shard, CH, 16, P_DIM)
    xpk = np.ascontiguousarray(
        np.transpose(xs, (2, 3, 1, 0)).reshape(128, CH * 8))
    return {"xpk": xpk}


def prep_shared_inputs(W_np):
    # w2[(i*8+p), n*16+d] = W[i, n, p, d]
    w2 = np.ascontiguousarray(
        np.transpose(W_np, (0, 2, 1, 3)).reshape(-1, ND).astype(np.float32))

    # mask0[sigma, b'] = 0.1 * (b' == b(sigma));  sigma = i'*8+b
    # maskc[sigma, n*8+b'] = (b' == b(sigma)); s-matmul output partition = n*8+b
    # xmask[q=(i'*8+p), i2*8+b] = (i2 == i')
    mask0 = np.zeros((128, 8), dtype=np.float32)
    maskc = np.zeros((128, 80), dtype=np.float32)
    e8 = np.zeros((8, 128), dtype=np.float32)
    xmask = np.zeros((128, 128), dtype=np.float32)
    for ip in range(16):
        for b in range(8):
            sig = ip * 8 + b
            mask0[sig, b] = 0.1
            maskc[sig, b::8] = 1.0
            e8[b, sig] = 1.0
    for ii in range(16):
        for p in range(P_DIM):
            xmask[ii * 8 + p, ii * 8:(ii + 1) * 8] = 1.0
    return {"w2": w2, "mask0": mask0, "maskc": maskc, "e8": e8,
            "xmask": xmask}


_NC_CACHE = {}
LAST_RESULT = None  # BassKernelResults of the most recent kernel() call


def _get_nc(I_dim=I_FULL):
    if I_dim not in _NC_CACHE:
        _NC_CACHE[I_dim] = build_nc(I_dim)
    return _NC_CACHE[I_dim]


def kernel(inputs: np.ndarray, W: np.ndarray, trace: bool = False) -> np.ndarray:
    global LAST_RESULT
    from concourse.bass_utils import run_bass_kernel_spmd

    inputs = np.asarray(inputs, dtype=np.float32)
    W = np.asarray(W, dtype=np.float32)
    B, I_dim, _ = inputs.shape

    nc = _get_nc(I_dim)
    shared = prep_shared_inputs(W)

    in_maps = []
    bs = B // N_CORES
    for c in range(N_CORES):
        m = dict(shared)
        m.update(prep_core_inputs(inputs[c * bs:(c + 1) * bs], I_dim))
        in_maps.append(m)

    res = run_bass_kernel_spmd(nc, in_maps, list(range(N_CORES)), trace=trace)
    LAST_RESULT = res
    outs = [res.results[c]["out"] for c in range(N_CORES)]
    v = np.concatenate(outs, axis=0)          # [64, 160]
    v = v.reshape(B, N_CAP, D_CAP)[:, :, None, :]   # [64, 10, 1, 16]
    return v.astype(np.float32)


# revision 50
# speedup vs baseline: 401.3504x; 401.3504x over previous
"""DigitCaps routing kernel for TRN2 (8 NeuronCores, SPMD data-parallel over batch).

Problem: inputs [64, 4096, 8] f32, W [4096, 10, 8, 16] f32.
  u_hat[b,i,n,d] = sum_p inputs[b,i,p] * W[i,n,p,d]
  3 dynamic-routing iterations (softmax over n, weighted sum over i, squash,
  agreement update), output v [64, 10, 1, 16] f32.

Sharding: batch 64 -> 8 cores x 8 samples. W replicated (streamed once per core).
All on-device data is f32: the routing fixed-point amplifies input rounding by
~100x, so fp16/bf16 anywhere on the value path fails accuracy.

Per-core device layout (sigma = i'*8 + b, where i' = i mod 16, b = local sample):
  U  [128=(i'*8+b), 256 chunks * 160]  f32   u_hat, chunk k holds i in [16k,16k+16)
  L  [128, 256*10] f32                       routing logits
Production: per chunk one matmul  lhsT = Xall_k [128=(i'*8+p), 128=(i'*8+b)]
  (block-diagonal x, prebuilt on host), rhs = W2_k [128=(i'*8+p), 160=(n,d)].
s-phase: per chunk matmul with lhsT = C_all [128, 80=(n,b)] (masked softmax
  weights) accumulating PSUM [80, 160]; diagonal extracted with 10 tiny DMAs.
Logit update: DVE mult by broadcast v + d-tree reduction, all f32.
"""

from contextlib import ExitStack

import numpy as np

import concourse.bass as bass
import concourse.tile as tile
from concourse import bacc, mybir
from concourse.tile import TileContext

AF = mybir.ActivationFunctionType
ALU = mybir.AluOpType

N_CORES = 8
B_FULL = 64
I_FULL = 4096
P_DIM = 8          # Din
N_CAP = 10
D_CAP = 16
ND = N_CAP * D_CAP  # 160
EPS = 1e-7
ROUTING_ITERS = 3

F32 = mybir.dt.float32


def build_nc(I_dim=I_FULL, b_shard=8, phases="all", repeat=1):
    """Build the single-core Bass program (SPMD: same program on all cores).

    phases: "all" | "prod" (production only) | "it0" (prod + uniform iter) |
            "s1" (.. + first s-iteration) — for cost attribution.
    repeat: run the whole pipeline N times back-to-back (for wall-clock
            timing: the per-repeat delta cancels dispatch overhead).
    """
    CH = I_dim // 16          # chunks of 16 capsules
    SUPC = 4                  # chunks per DMA superchunk
    SUP = CH // SUPC
    GRP = min(8, CH)          # chunks per group for DVE staging
    NG = CH // GRP

    # tiny SWDGE ring: all our DMAs go through HWDGE (sync/scalar)
    nc = bacc.Bacc(dynamic_dma_scratch_size=1024)

    w2_d = nc.dram_tensor("w2", [CH * 128, ND], F32, kind="ExternalInput")
    # xpk[q=(i'*8+p), k*8+b] = x[b, 16k+i', p] — compact per-chunk x operand
    xpk_d = nc.dram_tensor("xpk", [128, CH * 8], F32, kind="ExternalInput")
    # xmask[q, m=(i2*8+b)] = 1 if i2 == q//8 else 0 (block-diagonal selector)
    xmask_d = nc.dram_tensor("xmask", [128, 128], F32, kind="ExternalInput")
    mask0_d = nc.dram_tensor("mask0", [128, 8], F32, kind="ExternalInput")
    maskc_d = nc.dram_tensor("maskc", [128, 80], F32, kind="ExternalInput")
    e8_d = nc.dram_tensor("e8", [8, 128], F32, kind="ExternalInput")
    out_d = nc.dram_tensor("out", [b_shard, ND], F32, kind="ExternalOutput")

    with TileContext(nc) as tc, ExitStack() as ctx:
        # ---- pools ----
        pU = ctx.enter_context(tc.tile_pool(name="U", bufs=1))
        pL = ctx.enter_context(tc.tile_pool(name="L", bufs=1))
        pconst = ctx.enter_context(tc.tile_pool(name="const", bufs=1))
        pw2 = ctx.enter_context(tc.tile_pool(name="w2", bufs=2))
        pxa = ctx.enter_context(tc.tile_pool(name="xa", bufs=2))
        psm = ctx.enter_context(tc.tile_pool(name="sm", bufs=2))
        pprod = ctx.enter_context(tc.tile_pool(name="prod", bufs=2))
        pca = ctx.enter_context(tc.tile_pool(name="ca", bufs=2))
        psmall = ctx.enter_context(tc.tile_pool(name="small", bufs=1))
        ppsP = ctx.enter_context(tc.tile_pool(name="psP", bufs=3, space="PSUM"))
        ppsS = ctx.enter_context(tc.tile_pool(name="psS", bufs=1, space="PSUM"))
        ppsV = ctx.enter_context(tc.tile_pool(name="psV", bufs=1, space="PSUM"))

        # ---- persistent tiles ----
        U = pU.tile([128, CH * ND], F32)
        L = pL.tile([128, CH * N_CAP], F32)
        xpk_sb = pconst.tile([128, CH * 8], F32)
        xmask_sb = pconst.tile([128, 128], F32)
        mask0_sb = pconst.tile([128, 8], F32)
        maskc_sb = pconst.tile([128, 80], F32)
        e8_sb = pconst.tile([8, 128], F32)

        nc.sync.dma_start(xpk_sb[:], xpk_d[:])
        nc.sync.dma_start(xmask_sb[:], xmask_d[:])
        nc.sync.dma_start(mask0_sb[:], mask0_d[:])
        nc.sync.dma_start(maskc_sb[:], maskc_d[:])
        nc.sync.dma_start(e8_sb[:], e8_d[:])
        nc.vector.memset(L[:], 0.0)

        eps_b = pconst.tile([128, 1], F32)
        one_b = pconst.tile([128, 1], F32)
        nc.vector.memset(eps_b[:], EPS)
        nc.vector.memset(one_b[:], 1.0)

        # ---- phase A: u_hat production ----
        # per superchunk: DMA W2, build the block-diagonal X tile on-chip
        # (broadcast xpk against xmask), then SUPC matmuls; 3 chunk outputs
        # share one PSUM bank -> one batched copy per 3 MMs
        w2_r = w2_d.rearrange("(s c p) f -> s p c f", c=SUPC, p=128)
        xpk_r = xpk_sb.rearrange("p (s c b) -> p s c b", c=SUPC, b=8)

        def produce():
          ps = None
          for s in range(SUP):
            w2b = pw2.tile([128, SUPC * ND], F32)
            (nc.sync if s % 2 == 0 else nc.scalar).dma_start(
                w2b.rearrange("p (c f) -> p c f", c=SUPC), w2_r[s])
            xab = pxa.tile([128, SUPC * 128], F32)
            xeng = nc.vector
            xeng.tensor_tensor(
                xab.rearrange("p (c i b) -> p c i b", c=SUPC, b=8),
                xpk_r[:, s].unsqueeze(2).to_broadcast([128, SUPC, 16, 8]),
                xmask_sb.rearrange("p (i b) -> p i b", b=8)
                    .unsqueeze(1).to_broadcast([128, SUPC, 16, 8]),
                ALU.mult,
            )
            for c in range(SUPC):
                k = s * SUPC + c
                j = k % 3
                if j == 0:
                    ps = ppsP.tile([128, 3 * ND], F32)
                nc.tensor.matmul(
                    ps[:, j * ND:(j + 1) * ND],
                    xab[:, c * 128:(c + 1) * 128],
                    w2b[:, c * ND:(c + 1) * ND],
                    start=True, stop=True,
                )
                if j == 2 or k == CH - 1:
                    lo = k - j
                    if (k // 3) % 2 == 0:
                        nc.vector.tensor_copy(
                            U[:, lo * ND:(k + 1) * ND], ps[:, 0:(j + 1) * ND])
                    else:
                        nc.scalar.copy(
                            U[:, lo * ND:(k + 1) * ND], ps[:, 0:(j + 1) * ND])

        # ---- helpers ----
        def squash(in_ap, scale, copy_in=True):
            """v = squash(scale * in_ap), elementwise over [8, 160]."""
            if copy_in:
                s_sb = psmall.tile([8, ND], F32, tag="sq_s")
                nc.scalar.mul(s_sb[:], in_ap, scale)
            else:
                s_sb = in_ap  # already an SBUF tile, scale must be 1
            sq = psmall.tile([8, ND], F32, tag="sq_sq")
            nc.vector.tensor_mul(sq[:], s_sb[:], s_sb[:])
            den = psmall.tile([8, ND], F32, tag="sq_den")
            nc.scalar.activation(den[:], sq[:], AF.Identity, bias=one_b[0:8, :])
            nc.vector.reciprocal(den[:], den[:])
            rt = psmall.tile([8, ND], F32, tag="sq_rt")
            nc.scalar.activation(rt[:], sq[:], AF.Sqrt, bias=eps_b[0:8, :])
            nc.vector.reciprocal(rt[:], rt[:])
            nc.vector.tensor_mul(sq[:], sq[:], den[:])     # sq/(1+sq)
            nc.vector.tensor_mul(rt[:], sq[:], rt[:])      # * rsqrt(sq+eps)
            v_sb = psmall.tile([8, ND], F32, tag="sq_v")
            nc.vector.tensor_mul(v_sb[:], rt[:], s_sb[:])
            return v_sb

        def s_uniform():
            """s0 = 0.1 * sum_i u_hat -> [8, 160] psum accumulation."""
            s0_ps = ppsS.tile([8, ND], F32, tag="s_acc")
            for k in range(CH):
                nc.tensor.matmul(
                    s0_ps[:], mask0_sb[:], U[:, k * ND:(k + 1) * ND],
                    start=(k == 0), stop=(k == CH - 1),
                )
            return s0_ps

        def broadcast_v(v_sb):
            """v [8,160] -> vf [128,160] (replicated per sample block)."""
            vf_ps = ppsV.tile([128, ND], F32)
            nc.tensor.matmul(vf_ps[:], e8_sb[:], v_sb[:], start=True, stop=True)
            vf = psmall.tile([128, ND], F32, tag="vf")
            nc.vector.tensor_copy(vf[:], vf_ps[:])
            return vf

        def logit_update_group(eng, g, vf):
            """L[sigma, (k,n)] += sum_d U[sigma,(k,n,d)] * v for group g."""
            Lv = L.rearrange("p (a x) -> p a x", x=GRP * N_CAP)
            pr = pprod.tile([128, GRP * ND], F32, tag="pr")
            eng.tensor_tensor(
                pr.rearrange("p (k f) -> p k f", k=GRP),
                U[:, g * GRP * ND:(g + 1) * GRP * ND]
                    .rearrange("p (k f) -> p k f", k=GRP),
                vf[:].unsqueeze(1).to_broadcast([128, GRP, ND]),
                ALU.mult,
            )
            pv = pr.rearrange("p (a d) -> p a d", d=D_CAP)
            eng.tensor_tensor(pv[:, :, 0:8], pv[:, :, 0:8], pv[:, :, 8:16], ALU.add)
            eng.tensor_tensor(pv[:, :, 0:4], pv[:, :, 0:4], pv[:, :, 4:8], ALU.add)
            eng.tensor_tensor(pv[:, :, 0:2], pv[:, :, 0:2], pv[:, :, 2:4], ALU.add)
            eng.tensor_tensor(pv[:, :, 0:1], pv[:, :, 0:1], pv[:, :, 1:2], ALU.add)
            eng.tensor_tensor(Lv[:, g, :], Lv[:, g, :], pv[:, :, 0], ALU.add)

        def logit_update(v_sb):
            vf = broadcast_v(v_sb)
            for g in range(NG):
                eng = nc.vector
                logit_update_group(eng, g, vf)

        def softmax_smm_group(g, s_ps):
            """Group-local softmax + C_all build + the s accumulation MMs."""
            Lg = L.rearrange("p (g k n) -> p g k n", k=GRP, n=N_CAP)
            ee = psm.tile([128, GRP * N_CAP], F32, tag="ee")
            nc.scalar.activation(
                ee[:], Lg[:, g].rearrange("p k n -> p (k n)"), AF.Exp)
            eev = ee.rearrange("p (k n) -> p k n", n=N_CAP)
            den = psm.tile([128, GRP], F32, tag="den")
            nc.vector.tensor_reduce(
                den[:], eev, axis=mybir.AxisListType.X, op=ALU.add)
            rr = psm.tile([128, GRP], F32, tag="rr")
            nc.vector.reciprocal(rr[:], den[:])
            cc = psm.tile([128, GRP * N_CAP], F32, tag="cc")
            nc.vector.tensor_tensor(
                cc.rearrange("p (k n) -> p k n", n=N_CAP), eev,
                rr[:].unsqueeze(2).to_broadcast([128, GRP, N_CAP]),
                ALU.mult,
            )
            ca = pca.tile([128, GRP * 80], F32)
            nc.vector.tensor_tensor(
                ca.rearrange("p (k n b) -> p k n b", k=GRP, b=8),
                cc.rearrange("p (k n) -> p k n", n=N_CAP)
                    .unsqueeze(3).to_broadcast([128, GRP, N_CAP, 8]),
                maskc_sb.rearrange("p (n b) -> p n b", b=8)
                    .unsqueeze(1).to_broadcast([128, GRP, N_CAP, 8]),
                ALU.mult,
            )
            for kk in range(GRP):
                k = g * GRP + kk
                nc.tensor.matmul(
                    s_ps[:],
                    ca[:, kk * 80:(kk + 1) * 80],
                    U[:, k * ND:(k + 1) * ND],
                    start=(k == 0), stop=(k == CH - 1),
                )

        def s_iteration(v_prev=None):
            """If v_prev is given, fuse its logit-update per group with this
            iteration's softmax + s-matmuls (the PE chases the DVE/GPSIMD)."""
            vf = broadcast_v(v_prev) if v_prev is not None else None
            s_ps = ppsS.tile([80, ND], F32, tag="s_acc")
            for g in range(NG):
                if vf is not None:
                    eng = nc.vector
                    logit_update_group(eng, g, vf)
                softmax_smm_group(g, s_ps)
            return s_ps

        def extract_diag(s_ps):
            """[80,160] psum -> s_sb [8,160]: s_sb[b, n*16+d] = s_ps[n*8+b, n*16+d]."""
            # vf is dead once the s accumulation finished -> reuse its slot
            s80 = psmall.tile([80, ND], F32, tag="vf")
            nc.scalar.copy(s80[:], s_ps[:])
            s_sb = psmall.tile([8, ND], F32, tag="sq_s")
            for n in range(N_CAP):
                nc.sync.dma_start(
                    s_sb[0:8, n * D_CAP:(n + 1) * D_CAP],
                    s80[n * 8:(n + 1) * 8, n * D_CAP:(n + 1) * D_CAP],
                )
            return s_sb

        # ---- routing (logit-update of iter t fused into s-iteration t+1) ----
        for rep in range(repeat):
            if rep > 0:
                nc.vector.memset(L[:], 0.0)
            produce()
            if phases == "prod":
                v_sb = psmall.tile([8, ND], F32, tag="sq_v")
                nc.vector.tensor_copy(v_sb[:], U[0:8, 0:ND])
            elif phases == "it0":
                v_sb = squash(s_uniform()[:], 1.0)
                logit_update(v_sb)
            elif phases == "s1":
                v_sb = squash(s_uniform()[:], 1.0)
                v_sb = squash(extract_diag(s_iteration(v_sb))[:], 1.0,
                              copy_in=False)
            else:
                # the 1/N is baked into mask0
                v_sb = squash(s_uniform()[:], 1.0)
                for it in range(1, ROUTING_ITERS):
                    s_sb = extract_diag(s_iteration(v_sb))
                    v_sb = squash(s_sb[:], 1.0, copy_in=False)

            nc.sync.dma_start(out_d[:], v_sb[:])

    nc.compile()
    if not nc.is_finalized():
        nc.finalize()
    return nc


# ------------------------- host-side data prep -------------------------

def prep_core_inputs(x_shard, I_dim=I_FULL):
    """Per-core xpk from x_shard [8, I, 8] f32."""
    CH = I_dim // 16
    b_shard = x_shard.shape[0]
    assert b_shard == 8

    # xs[b, k, i', p] -> xpk[(i'*8+p), k*8+b]
    xs = x_shard.reshape(b_shard, CH, 16, P_DIM)
    xpk = np.ascontiguousarray(
        np.transpose(xs, (2, 3, 1, 0)).reshape(128, CH * 8))
    return {"xpk": xpk}


def prep_shared_inputs(W_np):
    # w2[(i*8+p), n*16+d] = W[i, n, p, d]
    w2 = np.ascontiguousarray(
        np.transpose(W_np, (0, 2, 1, 3)).reshape(-1, ND).astype(np.float32))

    # mask0[sigma, b'] = 0.1 * (b' == b(sigma));  sigma = i'*8+b
    # maskc[sigma, n*8+b'] = (b' == b(sigma)); s-matmul output partition = n*8+b
    # xmask[q=(i'*8+p), i2*8+b] = (i2 == i')
    mask0 = np.zeros((128, 8), dtype=np.float32)
    maskc = np.zeros((128, 80), dtype=np.float32)
    e8 = np.zeros((8, 128), dtype=np.float32)
    xmask = np.zeros((128, 128), dtype=np.float32)
    for ip in range(16):
        for b in range(8):
            sig = ip * 8 + b
            mask0[sig, b] = 0.1
            maskc[sig, b::8] = 1.0
            e8[b, sig] = 1.0
    for ii in range(16):
        for p in range(P_DIM):
            xmask[ii * 8 + p, ii * 8:(ii + 1) * 8] = 1.0
    return {"w2": w2, "mask0": mask0, "maskc": maskc, "e8": e8,
            "xmask": xmask}


_NC_CACHE = {}
LAST_RESULT = None  # BassKernelResults of the most recent kernel() call


def _get_nc(I_dim=I_FULL):
    if I_dim not in _NC_CACHE:
        _NC_CACHE[I_dim] = build_nc(I_dim)
    return _NC_CACHE[I_dim]


def kernel(inputs: np.ndarray, W: np.ndarray, trace: bool = False) -> np.ndarray:
    global LAST_RESULT
    from concourse.bass_utils import run_bass_kernel_spmd

    inputs = np.asarray(inputs, dtype=np.float32)
    W = np.asarray(W, dtype=np.float32)
    B, I_dim, _ = inputs.shape

    nc = _get_nc(I_dim)
    shared = prep_shared_inputs(W)

    in_maps = []
    bs = B // N_CORES
    for c in range(N_CORES):
        m = dict(shared)
        m.update(prep_core_inputs(inputs[c * bs:(c + 1) * bs], I_dim))
        in_maps.append(m)

    res = run_bass_kernel_spmd(nc, in_maps, list(range(N_CORES)), trace=trace)
    LAST_RESULT = res
    outs = [res.results[c]["out"] for c in range(N_CORES)]
    v = np.concatenate(outs, axis=0)          # [64, 160]
    v = v.reshape(B, N_CAP, D_CAP)[:, :, None, :]   # [64, 10, 1, 16]
    return v.astype(np.float32)
